# revision 26
# baseline (speedup 1.0000x reference)
"""Trainium2 Bass kernel for GCNN operator:
    h   = einsum('bnf,nfg->bng', x, kernel)   # per-node feature transform
    out = einsum('nm,bmg->bng', A, h) + bias  # dense adjacency aggregation

Sharding: node dim N row-sharded across 8 cores for the A @ H aggregation
(the memory-bound part: each core streams its A row-shard once). The tiny
per-node transform h (8.4M MACs) is computed REDUNDANTLY on every core's
VectorEngine for all N nodes — that costs ~9MiB of extra HBM reads (full
x/kernel instead of a shard) but removes the AllGather + its one-time ~50us
ncfw barrier from the critical path entirely, and h chunks become available
progressively so the TensorEngine starts within ~10us.

A^T is streamed as fp8 E3M4 (halves HBM bytes; measured rel-err ~1.25e-2
against the fp32 reference, gate is 2e-2). Matmuls are 4-way column-tiled
(tile_position=(0,32t)): fp16 stationary hq x fp8 moving A, 4 concurrent
MMs accumulating into a single PSUM bank [128, 512] that holds out^T in
(col-group, fused-bg-column) layout. Bias is pre-arranged on the host into
the same PSUM layout, so the epilogue is one DVE add + one store; the host
reassembles the [B, nl, G] block (free).

Self-contained: hardcodes shapes; only imports concourse + numpy/ml_dtypes.
"""

import numpy as np

B, N, F, G = 2, 16384, 16, 16
NCORES = 8
P = 128                    # SBUF partitions
C = B * G                  # 32 fused (batch, out-feature) columns
MC = 16                    # m-blocks per h-compute chunk
KM = 4                     # m-blocks per A-stream DMA tile


def build_nc(n=N, ncores=NCORES, at_bufs=14, km=KM):
    """Build the per-core Bass program (SPMD: same program on all cores)."""
    import concourse.bass as bass
    import concourse.mybir as mybir
    import concourse.tile as tile
    from concourse import bacc

    f32 = mybir.dt.float32
    f16 = mybir.dt.float16
    f8 = mybir.dt.float8e3

    nl = n // ncores           # nodes per core (output shard)
    j_n = nl // P              # local node blocks
    mj = n // P                # contraction blocks (128 at full size)
    ntc = max(nl // 4, 1)      # nodes per PSUM col-group (512 at full size)
    km = min(km, mj)           # contraction blocks batched per DMA
    mc = min(MC, mj)           # m-blocks per h chunk
    n_hc = mj // mc            # h chunks

    nc = bacc.Bacc(
        "TRN2", target_bir_lowering=False, debug=False, num_devices=ncores
    )

    # xs/ks are pre-arranged on the host into SBUF layout (partition-major,
    # contiguous per partition line) so each load is 128 clean descriptors.
    # ks is G-major ([.., G, F]) so every DVE operand is unit-stride.
    at = nc.dram_tensor("at", [mj // km, P, km, nl], f8, kind="ExternalInput")
    xs = nc.dram_tensor("xs", [P, mj, B, F], f16, kind="ExternalInput")
    ks = nc.dram_tensor("ks", [n_hc, P, mc, G, F], f16, kind="ExternalInput")
    bs = nc.dram_tensor("bs", [P, ntc], f32, kind="ExternalInput")
    outs = nc.dram_tensor("outs", [P, ntc], f32, kind="ExternalOutput")

    with tile.TileContext(nc) as tc:
        with (
            tc.tile_pool(name="const", bufs=1) as const,
            tc.tile_pool(name="kp", bufs=4) as kp,
            tc.tile_pool(name="work", bufs=2) as work,
            tc.tile_pool(name="atp", bufs=at_bufs) as atp,
            tc.tile_pool(name="pacc", bufs=1, space="PSUM") as pacc,
        ):
            # ---- prologue loads: all on the two HWDGE queues, front-loaded
            #      ahead of / interleaved with the A stream (the SWDGE queue
            #      gets starved to ~60GB/s when the stream saturates HBM) ----
            x_full = const.tile([P, mj, B, F], f16)
            nc.scalar.dma_start(out=x_full[:, :, :, :], in_=xs.ap())
            bias_sb = const.tile([P, ntc], f32)
            nc.scalar.dma_start(out=bias_sb[:, :], in_=bs.ap())

            # ---- h = einsum('bnf,nfg->bng') on DVE (fp16), ALL nodes,
            #      chunked so k-load / h-compute / matmuls pipeline ----
            # hq[p, m, b, g] = h[b, m*P+p, g]; lhsT slice hq[:, m, :, :] is
            # the [128, C] stationary operand for contraction block m.
            hq = const.tile([P, mj, B, G], f16)
            ks_r = ks.ap()
            k_tiles = []
            k_t = kp.tile([P, mc, G, F], f16, tag="k_t", name="k_t")
            nc.sync.dma_start(out=k_t[:, :, :, :], in_=ks_r[0])
            k_tiles.append(k_t)

            def h_chunk(cc):
                # TENSOR_REDUCE runs at 1x; tensor_tensor adds run at 2x for
                # fp16, so fold F 16->8->4 with adds before the final reduce.
                sl = slice(cc * mc, (cc + 1) * mc)
                k_t = k_tiles[cc]
                for b in range(B):
                    prod = work.tile([P, mc, G, F], f16, tag="prod")
                    nc.vector.tensor_tensor(
                        prod[:, :, :, :],
                        x_full[:, sl, b, None, :].to_broadcast([P, mc, G, F]),
                        k_t[:, :, :, :],
                        mybir.AluOpType.mult,
                    )
                    f8t = work.tile([P, mc, G, F // 2], f16, tag="f8t")
                    nc.vector.tensor_tensor(
                        f8t[:, :, :, :],
                        prod[:, :, :, 0 : F // 2],
                        prod[:, :, :, F // 2 : F],
                        mybir.AluOpType.add,
                    )
                    f4t = work.tile([P, mc, G, F // 4], f16, tag="f4t")
                    nc.vector.tensor_tensor(
                        f4t[:, :, :, :],
                        f8t[:, :, :, 0 : F // 4],
                        f8t[:, :, :, F // 4 : F // 2],
                        mybir.AluOpType.add,
                    )
                    nc.vector.tensor_reduce(
                        hq[:, sl, b, :],
                        f4t[:, :, :, :],
                        axis=mybir.AxisListType.X,
                        op=mybir.AluOpType.add,
                    )

            # ---- main loop: 4-way col-tiled out^T accumulation ----
            # One PSUM bank [P, ntc] f32: partition 32t + c holds
            # out^T[c, t*ntc + i] (c = b*G+g fused column, i free index).
            # Remaining k chunks are interleaved into the head of the A
            # stream; h chunk cc is emitted (DVE) as soon as its k tile's
            # load is queued, keeping hq production ahead of the matmuls.
            acc = pacc.tile([P, ntc], f32, tag="acc", name="acc")
            at_stream = at.ap()
            tiles_per_hc = max(mc // km, 1)
            with nc.allow_low_precision(reason="16-term fp16 sum, err << fp8 A"):
                for mb in range(mj // km):
                    eng = nc.scalar if mb % 2 else nc.sync
                    cc = mb // tiles_per_hc
                    if mb % tiles_per_hc == 0:
                        if cc + 1 < n_hc:
                            k_t = kp.tile([P, mc, G, F], f16, tag="k_t", name="k_t")
                            eng.dma_start(out=k_t[:, :, :, :], in_=ks_r[cc + 1])
                            k_tiles.append(k_t)
                        if cc < n_hc:
                            h_chunk(cc)
                    at_t = atp.tile([P, km, nl], f8, tag="at_t", name="at_t")
                    eng.dma_start(out=at_t[:, :, :], in_=at_stream[mb])
                    for kk in range(km):
                        m = mb * km + kk
                        for t in range(4):
                            nc.tensor.matmul(
                                acc[32 * t : 32 * (t + 1), :],
                                hq[:, m, :, :],
                                at_t[:, kk, t * ntc : (t + 1) * ntc],
                                start=(m == 0),
                                stop=(m == mj - 1),
                                tile_position=(0, 32 * t),
                            )

            # ---- epilogue: out = acc + bias (both already in PSUM layout) ----
            out_sb = work.tile([P, ntc], f32, tag="out_sb")
            nc.vector.tensor_tensor(
                out_sb[:, :], acc[:, :], bias_sb[:, :], mybir.AluOpType.add
            )
            nc.sync.dma_start(out=outs.ap(), in_=out_sb[:, :])

    nc.compile()
    return nc


_NC_CACHE = {}


def _get_nc(n=N, ncores=NCORES):
    key = (n, ncores)
    if key not in _NC_CACHE:
        _NC_CACHE[key] = build_nc(n, ncores)
    return _NC_CACHE[key]


def make_in_maps(x, A, kern, bias, n=N, ncores=NCORES, km=KM):
    import ml_dtypes

    f8 = ml_dtypes.float8_e3m4
    nl = n // ncores
    mj = n // P
    km = min(km, mj)
    ntc = max(nl // 4, 1)
    mc = min(MC, mj)
    n_hc = mj // mc
    # xs[p, m, b, f] = x[b, m*P+p, f]
    x16 = np.ascontiguousarray(
        x.reshape(B, mj, P, F).transpose(2, 1, 0, 3)
    ).astype(np.float16)
    # ks[cc, p, m, g, f] = kern[(cc*mc+m)*P + p, f, g] (G-major for DVE)
    k16 = np.ascontiguousarray(
        kern.reshape(n_hc, mc, P, F, G).transpose(0, 2, 1, 4, 3)
    ).astype(np.float16)
    in_maps = []
    for r in range(ncores):
        sl = slice(r * nl, (r + 1) * nl)
        # at[mb, p, kk, j] = A[sl.start + j, (mb*km + kk)*P + p], fp8 e3m4
        at = np.ascontiguousarray(A[sl, :].T)  # [n, nl]
        at = at.reshape(mj // km, km, P, nl).transpose(0, 2, 1, 3)
        at = np.ascontiguousarray(at).astype(f8)
        # bias in PSUM layout: bs[32t + b*G + g, i] = bias[sl.start + t*ntc + i, g]
        bl = bias[sl].reshape(4, ntc, G).transpose(0, 2, 1)  # [t, g, i]
        bs = np.ascontiguousarray(
            np.broadcast_to(bl[:, None], (4, B, G, ntc)).reshape(P, ntc)
        ).astype(np.float32)
        in_maps.append({"at": at, "xs": x16, "ks": k16, "bs": bs})
    return in_maps


def assemble_out(results, n=N, ncores=NCORES):
    nl = n // ncores
    ntc = max(nl // 4, 1)
    parts = []
    for r in range(ncores):
        o = results[r]["outs"].reshape(4, B, G, ntc)
        # out[b, t*ntc + i, g] = o[t, b, g, i]
        blk = np.ascontiguousarray(o.transpose(1, 0, 3, 2)).reshape(B, nl, G)
        parts.append(blk)
    return np.ascontiguousarray(np.concatenate(parts, axis=1))


def run(inputs, n=N, ncores=NCORES, trace=False, **spmd_kwargs):
    from concourse.bass_utils import run_bass_kernel_spmd

    x = np.asarray(inputs["x"], dtype=np.float32)
    A = np.asarray(inputs["A"], dtype=np.float32)
    kern = np.asarray(inputs["kernel"], dtype=np.float32)
    bias = np.asarray(inputs["bias"], dtype=np.float32)
    nc = _get_nc(n, ncores)
    in_maps = make_in_maps(x, A, kern, bias, n, ncores)
    res = run_bass_kernel_spmd(
        nc, in_maps, list(range(ncores)), trace=trace, **spmd_kwargs
    )
    out = assemble_out(res.results, n, ncores)
    return out, res


def kernel(**inputs) -> np.ndarray:
    out, _ = run(inputs)
    return out
